# revision 4
# baseline (speedup 1.0000x reference)
"""Contrastive Predictive Coding loss kernel for 8 Trainium2 NeuronCores.

Strategy (SPMD, batch row-sharded):
  - B=8192 rows split across 8 cores (1024 rows each). All activations are
    kept TRANSPOSED on chip ([D, batch] with D on partitions) so every matmul
    uses the torch-layout [in,out] weights directly as lhsT with no on-device
    transposes; the raw inputs are transposed (and bf16-cast) on host.
  - Softmax shift-invariance: logits_ij = 10*pred_i.(h1_j @ W2 + b2); the b2
    term is a per-row constant that cancels exactly in LSE_i - logit_ii, and
    pred.(h1 @ W2) = (pred @ W2^T).h1, so the encoder's second layer for
    next_state is never materialized. Each core computes q = pred @ W2^T
    (W2^T supplied by host) and contracts logits against the relu layer-1
    output h1, which every core computes for the FULL batch (redundant
    layer-1 encode is cheaper than an AllGather here).
  - Logits row-block (1024 x 8192) accumulates in PSUM from bf16 matmuls;
    softmax statistics are fused into the ScalarE pass: exp(10*x) with
    accum_out row-sums, no max-subtraction (logits are bounded ~|8|, exp is
    fp32-safe). The final log() runs on host (8192 values).
  - diag(logits) = rowsum(q * h1_local) via fp32 elementwise multiply + a
    matmul against a constant 10.0 vector (partition reduction on PE).
  - Per-row [rowsum, diag] returned as [128, 16] per core; host finishes
    loss = mean(log(rowsum) - diag).
"""

import os
import sys

import numpy as np

for _p in ("/opt/trn_rl_repo", "/root/.axon_site/_ro/trn_rl_repo"):
    if os.path.isdir(_p) and _p not in sys.path:
        sys.path.append(_p)

D = 256
B = 8192
NCORES = 8
SH = B // NCORES          # 1024 rows per core
KP = D // 128             # 2 partition chunks of the contraction dim
NB = 512                  # matmul moving-operand block (one fp32 PSUM bank)
RT = SH // 128            # 8 row tiles of 128 pred rows
G = 2048                  # columns per fused exp/accum group (4 PSUM banks)
NG = B // G               # 4 groups per row tile
SCALE = 10.0              # 1 / temperature

_cache = {}


def _build():
    if "nc" in _cache:
        return _cache["nc"]

    import concourse.bacc as bacc
    import concourse.mybir as mybir
    import concourse.tile as tile

    dt = mybir.dt
    f32 = dt.float32
    bf16 = dt.bfloat16
    AF = mybir.ActivationFunctionType
    ALU = mybir.AluOpType
    AX = mybir.AxisListType

    nc = bacc.Bacc("TRN2", target_bir_lowering=False, num_devices=NCORES)

    xsT = nc.declare_dram_parameter("xsT", [D, SH], bf16, isOutput=False)
    xnTl = nc.declare_dram_parameter("xnTl", [D, SH], bf16, isOutput=False)
    xnTf = nc.declare_dram_parameter("xnTf", [D, B], bf16, isOutput=False)
    w_e1 = nc.declare_dram_parameter("w_e1", [D, D], bf16, isOutput=False)
    w_e2 = nc.declare_dram_parameter("w_e2", [D, D], bf16, isOutput=False)
    w_e2T = nc.declare_dram_parameter("w_e2T", [D, D], bf16, isOutput=False)
    w_g = nc.declare_dram_parameter("w_g", [D, 3 * D], bf16, isOutput=False)
    w_p1 = nc.declare_dram_parameter("w_p1", [D, D], bf16, isOutput=False)
    w_p2 = nc.declare_dram_parameter("w_p2", [D, D], bf16, isOutput=False)
    # bias columns: 0=enc_b1 1=enc_b2 2=gru_r 3=gru_z 4=gru_n(bih) 5=gru_n(bhh)
    #               6=pred_b1 7=pred_b2
    bpk = nc.declare_dram_parameter("bpk", [D, 8], f32, isOutput=False)
    # out columns: 0..7 = exp-row-sums per row tile, 8..15 = diag10 per tile
    out_d = nc.declare_dram_parameter("out", [128, 2 * RT], f32, isOutput=True)

    with tile.TileContext(nc, num_cores=NCORES) as tc:
        with (
            tc.tile_pool(name="persist", bufs=1) as pp,
            tc.tile_pool(name="scratch", bufs=8) as sp,
            tc.tile_pool(name="small", bufs=4) as smp,
        ):
            # ---- small inputs first (gate the local compute chain) -------
            xs, xnl = [], []
            for k in range(KP):
                t = pp.tile([128, SH], bf16, name=f"xs_{k}")
                nc.sync.dma_start(out=t[:, :], in_=xsT[k * 128 : (k + 1) * 128, :])
                xs.append(t)
                t = pp.tile([128, SH], bf16, name=f"xnl_{k}")
                nc.sync.dma_start(out=t[:, :], in_=xnTl[k * 128 : (k + 1) * 128, :])
                xnl.append(t)
            wt = {}
            for name, hdl, width in (
                ("we1", w_e1, D),
                ("we2", w_e2, D),
                ("wg", w_g, 3 * D),
                ("wp1", w_p1, D),
                ("wp2", w_p2, D),
                ("we2T", w_e2T, D),
            ):
                wt[name] = []
                for k in range(KP):
                    t = pp.tile([128, width], bf16, name=f"{name}_{k}")
                    nc.sync.dma_start(out=t[:, :], in_=hdl[k * 128 : (k + 1) * 128, :])
                    wt[name].append(t)
            bp = []
            for k in range(KP):
                t = pp.tile([128, 8], f32, name=f"bp_{k}")
                nc.sync.dma_start(out=t[:, :], in_=bpk[k * 128 : (k + 1) * 128, :])
                bp.append(t)
            tens = pp.tile([128, 1], f32, name="tens")
            nc.vector.memset(tens[:, :], SCALE)

            # full next_state, chunked so layer-1 can start on chunk 0
            xnf = [pp.tile([128, B], bf16, name=f"xnf_{k}") for k in range(KP)]
            XCH = 2048
            for c0 in range(0, B, XCH):
                for k in range(KP):
                    nc.sync.dma_start(
                        out=xnf[k][:, c0 : c0 + XCH],
                        in_=xnTf[k * 128 : (k + 1) * 128, c0 : c0 + XCH],
                    )

            # persistent activations
            h1f = [pp.tile([128, B], bf16, name=f"h1f_{k}") for k in range(KP)]
            qT = [pp.tile([128, SH], bf16, name=f"qT_{k}") for k in range(KP)]
            dummy = pp.tile([128, G], f32, name="dummy")
            out_sb = pp.tile([128, 2 * RT], f32, name="out_sb")

            with tc.tile_pool(name="ps1", bufs=3, space="PSUM") as ps1:

                def layer(dst, src, w, bias_col, act, tag, width=SH):
                    """dst[m][:, :width] = act(w.T @ src + b); bias_col=None
                    for a pure copy epilogue."""
                    for m in range(KP):
                        for g0 in range(0, width, SH):
                            ps = ps1.tile(
                                [128, SH], f32, name=f"ps_{tag}_{m}_{g0}", tag="p1"
                            )
                            for nb in range(SH // NB):
                                sl = slice(nb * NB, (nb + 1) * NB)
                                gsl = slice(g0 + nb * NB, g0 + (nb + 1) * NB)
                                for k in range(KP):
                                    nc.tensor.matmul(
                                        ps[:, sl],
                                        lhsT=w[k][:, m * 128 : (m + 1) * 128],
                                        rhs=src[k][:, gsl],
                                        start=(k == 0),
                                        stop=(k == KP - 1),
                                    )
                            dsl = slice(g0, g0 + SH)
                            if act == "relu":
                                nc.vector.tensor_scalar(
                                    out=dst[m][:, dsl],
                                    in0=ps[:, :],
                                    scalar1=bp[m][:, bias_col : bias_col + 1],
                                    scalar2=0.0,
                                    op0=ALU.add,
                                    op1=ALU.max,
                                )
                            elif act == "bias":
                                nc.vector.tensor_scalar(
                                    out=dst[m][:, dsl],
                                    in0=ps[:, :],
                                    scalar1=bp[m][:, bias_col : bias_col + 1],
                                    scalar2=None,
                                    op0=ALU.add,
                                )
                            else:  # pure copy
                                nc.vector.tensor_copy(dst[m][:, dsl], ps[:, :])

                # ---- local chain: z_t -> GRU -> pred -> q ----------------
                h1s = [sp.tile([128, SH], bf16, name=f"h1s_{k}", tag="scr") for k in range(KP)]
                zt = [sp.tile([128, SH], bf16, name=f"zt_{k}", tag="scr") for k in range(KP)]
                layer(h1s, xs, wt["we1"], 0, "relu", "h1s")
                layer(zt, h1s, wt["we2"], 1, "bias", "zt")

                def gate_psum(gate, m, tag):
                    ps = ps1.tile([128, SH], f32, name=f"ps_{tag}_{m}", tag="p1")
                    col0 = gate * D + m * 128
                    for nb in range(SH // NB):
                        sl = slice(nb * NB, (nb + 1) * NB)
                        for k in range(KP):
                            nc.tensor.matmul(
                                ps[:, sl],
                                lhsT=wt["wg"][k][:, col0 : col0 + 128],
                                rhs=zt[k][:, sl],
                                start=(k == 0),
                                stop=(k == KP - 1),
                            )
                    return ps

                rg = [sp.tile([128, SH], f32, name=f"rg_{m}", tag="scr") for m in range(KP)]
                zg = [sp.tile([128, SH], f32, name=f"zg_{m}", tag="scr") for m in range(KP)]
                ng = [sp.tile([128, SH], f32, name=f"ng_{m}", tag="scr") for m in range(KP)]
                ctx = [sp.tile([128, SH], bf16, name=f"ctx_{m}", tag="scr") for m in range(KP)]
                for m in range(KP):
                    ps_r = gate_psum(0, m, "gr")
                    nc.scalar.activation(
                        out=rg[m][:, :], in_=ps_r[:, :], func=AF.Sigmoid,
                        bias=bp[m][:, 2:3],
                    )
                for m in range(KP):
                    ps_z = gate_psum(1, m, "gz")
                    nc.scalar.activation(
                        out=zg[m][:, :], in_=ps_z[:, :], func=AF.Sigmoid,
                        bias=bp[m][:, 3:4],
                    )
                for m in range(KP):
                    ps_n = gate_psum(2, m, "gn")
                    tmp = sp.tile([128, SH], f32, name=f"tmp_{m}", tag="scr")
                    nc.vector.tensor_scalar(
                        out=tmp[:, :], in0=rg[m][:, :],
                        scalar1=bp[m][:, 5:6], scalar2=None, op0=ALU.mult,
                    )
                    nc.vector.tensor_tensor(
                        out=tmp[:, :], in0=ps_n[:, :], in1=tmp[:, :], op=ALU.add
                    )
                    nc.scalar.activation(
                        out=ng[m][:, :], in_=tmp[:, :], func=AF.Tanh,
                        bias=bp[m][:, 4:5],
                    )
                    omz = sp.tile([128, SH], f32, name=f"omz_{m}", tag="scr")
                    nc.vector.tensor_scalar(
                        out=omz[:, :], in0=zg[m][:, :],
                        scalar1=-1.0, scalar2=1.0, op0=ALU.mult, op1=ALU.add,
                    )
                    nc.vector.tensor_tensor(
                        out=ctx[m][:, :], in0=omz[:, :], in1=ng[m][:, :], op=ALU.mult
                    )

                h1p = [sp.tile([128, SH], bf16, name=f"h1p_{k}", tag="scr") for k in range(KP)]
                pr = [sp.tile([128, SH], bf16, name=f"pr_{k}", tag="scr") for k in range(KP)]
                layer(h1p, ctx, wt["wp1"], 6, "relu", "h1p")
                layer(pr, h1p, wt["wp2"], 7, "bias", "pr")
                # q = pred @ W2^T  (no bias: enc_b2 cancels in LSE - diag)
                layer(qT, pr, wt["we2T"], None, "copy", "q")

                # ---- diag: 10 * rowsum(q * h1_local) ---------------------
                h1nl = [sp.tile([128, SH], bf16, name=f"h1nl_{k}", tag="scr") for k in range(KP)]
                layer(h1nl, xnl, wt["we1"], 0, "relu", "h1nl")
                dp = ps1.tile([128, RT], f32, name="dp", tag="dp", bufs=1)
                prod = [sp.tile([128, SH], f32, name=f"prod_{k}", tag="scr") for k in range(KP)]
                for k in range(KP):
                    nc.vector.tensor_tensor(
                        out=prod[k][:, :], in0=qT[k][:, :], in1=h1nl[k][:, :],
                        op=ALU.mult,
                    )
                for t in range(RT):
                    for k in range(KP):
                        nc.tensor.matmul(
                            dp[:, t : t + 1],
                            lhsT=prod[k][:, t * 128 : (t + 1) * 128],
                            rhs=tens[:, :],
                            start=(k == 0),
                            stop=(k == KP - 1),
                        )
                nc.vector.tensor_copy(out_sb[:, RT : 2 * RT], dp[:, :])

                # ---- layer-1 for the FULL batch (redundant encode) -------
                layer(h1f, xnf, wt["we1"], 0, "relu", "h1f", width=B)

            # ---- logits + fused softmax statistics ----------------------
            with tc.tile_pool(name="psL", bufs=2, space="PSUM") as psL:
                for t in range(RT):
                    sums = smp.tile([128, NG], f32, name="sums", tag="sums")
                    for g in range(NG):
                        pl = psL.tile([128, G], f32, name="pl", tag="pl")
                        for s in range(G // NB):
                            c0 = g * G + s * NB
                            sl = slice(s * NB, (s + 1) * NB)
                            for k in range(KP):
                                nc.tensor.matmul(
                                    pl[:, sl],
                                    lhsT=qT[k][:, t * 128 : (t + 1) * 128],
                                    rhs=h1f[k][:, c0 : c0 + NB],
                                    start=(k == 0),
                                    stop=(k == KP - 1),
                                )
                        nc.scalar.activation(
                            out=dummy[:, :], in_=pl[:, :], func=AF.Exp,
                            scale=SCALE, accum_out=sums[:, g : g + 1],
                        )
                    nc.vector.reduce_sum(
                        out=out_sb[:, t : t + 1], in_=sums[:, :], axis=AX.X
                    )

            nc.sync.dma_start(out=out_d[:, :], in_=out_sb[:, :])

    if not nc.is_finalized():
        nc.finalize()
    _cache["nc"] = nc
    return nc


def _prep_in_maps(inputs):
    import ml_dtypes

    bf = ml_dtypes.bfloat16
    f = lambda x: np.ascontiguousarray(np.asarray(x), dtype=np.float32)
    state = f(inputs["state"])
    next_state = f(inputs["next_state"])
    bias_pack = np.stack(
        [
            f(inputs["enc_b1"]),
            f(inputs["enc_b2"]),
            f(inputs["gru_bih"])[:D] + f(inputs["gru_bhh"])[:D],
            f(inputs["gru_bih"])[D : 2 * D] + f(inputs["gru_bhh"])[D : 2 * D],
            f(inputs["gru_bih"])[2 * D :],
            f(inputs["gru_bhh"])[2 * D :],
            f(inputs["pred_b1"]),
            f(inputs["pred_b2"]),
        ],
        axis=1,
    )
    w_e2 = f(inputs["enc_w2"])
    nT = np.ascontiguousarray(next_state.T).astype(bf)  # [D, B]
    sT = np.ascontiguousarray(state.T).astype(bf)
    shared = {
        "w_e1": f(inputs["enc_w1"]).astype(bf),
        "w_e2": w_e2.astype(bf),
        "w_e2T": np.ascontiguousarray(w_e2.T).astype(bf),
        "w_g": f(inputs["gru_wih"]).astype(bf),
        "w_p1": f(inputs["pred_w1"]).astype(bf),
        "w_p2": f(inputs["pred_w2"]).astype(bf),
        "bpk": np.ascontiguousarray(bias_pack, dtype=np.float32),
        "xnTf": nT,
    }
    in_maps = []
    for c in range(NCORES):
        sl = slice(c * SH, (c + 1) * SH)
        in_maps.append(
            {
                "xsT": np.ascontiguousarray(sT[:, sl]),
                "xnTl": np.ascontiguousarray(nT[:, sl]),
                **shared,
            }
        )
    return in_maps


last_results = None


def _finish(results):
    total = 0.0
    for r in results:
        o = r["out"].astype(np.float64)
        rowsum = o[:, :RT]
        diag10 = o[:, RT:]
        total += float((np.log(rowsum) - diag10).sum())
    return np.float32(total / B)


def kernel(**inputs) -> np.ndarray:
    from concourse.bass_utils import run_bass_kernel_spmd

    global last_results
    nc = _build()
    in_maps = _prep_in_maps(inputs)
    res = run_bass_kernel_spmd(nc, in_maps, core_ids=list(range(NCORES)))
    last_results = res
    return _finish(res.results)


# ---------------------------------------------------------------------------
# Pure-numpy golden model of the exact device algorithm (for test.py).
def golden(**inputs) -> np.ndarray:
    in_maps = _prep_in_maps(inputs)
    f32 = np.float32
    m0 = in_maps[0]
    bfd = m0["w_e1"].dtype

    def as32(x):
        return x.astype(f32)

    # full layer-1 (same on every core)
    h1f = np.maximum(as32(m0["xnTf"]).T @ as32(m0["w_e1"]) + m0["bpk"][:, 0], 0.0)
    h1f = h1f.astype(bfd).astype(f32)  # [B, D]
    results = []
    for c in range(NCORES):
        m = in_maps[c]
        xs = as32(m["xsT"]).T
        h1 = np.maximum(xs @ as32(m["w_e1"]) + m["bpk"][:, 0], 0.0)
        h1 = h1.astype(bfd).astype(f32)
        zt = (h1 @ as32(m["w_e2"]) + m["bpk"][:, 1]).astype(bfd).astype(f32)
        gi = zt @ as32(m["w_g"])
        r = 1.0 / (1.0 + np.exp(-(gi[:, :D] + m["bpk"][:, 2])))
        z = 1.0 / (1.0 + np.exp(-(gi[:, D : 2 * D] + m["bpk"][:, 3])))
        n = np.tanh(gi[:, 2 * D :] + m["bpk"][:, 4] + r * m["bpk"][:, 5])
        ctx = ((1.0 - z) * n).astype(bfd).astype(f32)
        h1p = np.maximum(ctx @ as32(m["w_p1"]) + m["bpk"][:, 6], 0.0)
        h1p = h1p.astype(bfd).astype(f32)
        pred = (h1p @ as32(m["w_p2"]) + m["bpk"][:, 7]).astype(bfd).astype(f32)
        q = (pred @ as32(m["w_e2T"])).astype(bfd).astype(f32)  # [SH, D]
        logits = SCALE * (q @ h1f.T)  # [SH, B]
        rowsum = np.exp(logits).sum(axis=1)
        h1l = np.maximum(as32(m["xnTl"]).T @ as32(m["w_e1"]) + m["bpk"][:, 0], 0.0)
        h1l = h1l.astype(bfd).astype(f32)
        diag10 = SCALE * (q * h1l).sum(axis=1)
        out = np.stack(
            [rowsum.reshape(RT, 128).T, diag10.reshape(RT, 128).T], axis=0
        )
        results.append(
            {"out": np.concatenate([out[0], out[1]], axis=1).astype(np.float32)}
        )
    return _finish(results)
